# revision 1
# baseline (speedup 1.0000x reference)
"""Trainium2 Bass kernel for nn_Decoder_22196390985918 (SPADE-style decoder).

Sharding: 8 cores = (batch b in 0..3) x (H-half in 0..1). Each core computes
out[b, :, h0:h0+64, :] for h0 = 64*(core%2).

Key algorithmic transform: the [B, 512, H, W] "middle" tensor (masked scatter
of per-region style vectors mu[b,j,:]) is never materialized. Since
middle[b,:,h,w] = mu[b, j*(h,w), :] with j* the last active region,
conv(middle) collapses to a conv over the 5 one-hot region masks sel_j with
per-batch tap tables G[j, cc, tap] = sum_k Wconv[cc, k, tap] * mu[b, j, k].
That turns ~77 GFLOP of 512-channel convs into one K=45 matmul per tile.

The SPADE branch (mask -> shared 3x3 conv -> relu -> gamma/beta 3x3 convs) is
computed directly: shared conv via K=27 im2col, gamma/beta convs as 9
accumulating K=128 taps with gamma and beta fused into one M=128 output.
The sigmoid blending factors are folded into the conv weights and biases.

All conv/table matmuls run in float32r (TF32-like); everything else is fp32.
Each im2col is built by a single multi-dim-AP DMA per output chunk; DMA
issue is spread across the sync/tensor/scalar/gpsimd queues.
"""
import os as _os

import numpy as np

import concourse.bacc as bacc
import concourse.bass as bass
import concourse.mybir as mybir
import concourse.tile as tile
from concourse.bass_utils import run_bass_kernel_spmd

dt = mybir.dt
F32 = dt.float32
F32R = dt.float32 if _os.environ.get("KF32") == "1" else dt.float32r
AF = mybir.ActivationFunctionType
ALU = mybir.AluOpType

B, C, H, W, F, L, NH = 4, 64, 128, 128, 5, 512, 128
GW = 130                    # padded grid width  (image col = grid col - 1)
SR = 66                     # seg/sel/actv grid rows (image row = h0 - 1 + r)
MR = 68                     # mask grid rows (image row = h0 - 2 + r)
SEG_N = SR * GW             # 8580
MASK_N = MR * GW            # 8840
SEG_SZ = SEG_N + 2 * GW + 2 + 520   # sel tail slack for im2col windows
MASK_SZ = MASK_N + 2 * GW + 2 + 390
ROWS = 64                   # output rows per core
NCH = 16                    # main conv chunks (4 rows x 128 cols, N=512)
ACH = 22                    # shared conv chunks (3 rows x 128 cols, N=384)
NCORES = 8


def _win_ap(base_ap, flat):
    """9-tap im2col source view: partitions from base_ap, free dims
    (ty[3] x tx[3] x flat window) as overlapping strided windows."""
    return bass.AP(tensor=base_ap.tensor, offset=base_ap.offset,
                   ap=[base_ap.ap[0], [GW, 3], [1, 3], [1, flat]])


def _build_nc():
    lvl = int(_os.environ.get("KSEC", "8"))
    nc = bacc.Bacc()

    # ---- per-core DRAM inputs -------------------------------------------
    xb = nc.dram_tensor("xb", [C, H * W], F32, kind="ExternalInput")
    xown = nc.dram_tensor("xown", [C, ROWS * W], F32, kind="ExternalInput")
    segg = nc.dram_tensor("segg", [F, SEG_N + 264], F32, kind="ExternalInput")
    maskg = nc.dram_tensor("maskg", [3, MASK_N + 264], F32, kind="ExternalInput")
    codes = nc.dram_tensor("codes", [F, L], F32, kind="ExternalInput")
    fcw = nc.dram_tensor("fcw", [F, L, L], F32, kind="ExternalInput")
    fcbt = nc.dram_tensor("fcbt", [L, F], F32, kind="ExternalInput")
    cgw = nc.dram_tensor("cgw", [C, L * 9], F32, kind="ExternalInput")
    cbw = nc.dram_tensor("cbw", [C, L * 9], F32, kind="ExternalInput")
    sgw = nc.dram_tensor("sgw", [C, NH * 9], F32, kind="ExternalInput")
    sbw = nc.dram_tensor("sbw", [C, NH * 9], F32, kind="ExternalInput")
    ssw = nc.dram_tensor("ssw", [NH, 27], F32, kind="ExternalInput")
    cgb = nc.dram_tensor("cgb", [C, 1], F32, kind="ExternalInput")
    cbb = nc.dram_tensor("cbb", [C, 1], F32, kind="ExternalInput")
    sgbb = nc.dram_tensor("sgbb", [C, 1], F32, kind="ExternalInput")
    sbbb = nc.dram_tensor("sbbb", [C, 1], F32, kind="ExternalInput")
    ssb = nc.dram_tensor("ssb", [NH, 1], F32, kind="ExternalInput")
    bg = nc.dram_tensor("bg", [1, 1], F32, kind="ExternalInput")
    bb = nc.dram_tensor("bb", [1, 1], F32, kind="ExternalInput")
    u5 = nc.dram_tensor("u5", [45, 45], F32, kind="ExternalInput")
    ident = nc.dram_tensor("ident", [128, 128], F32, kind="ExternalInput")
    zz = nc.dram_tensor("zz", [128, 652], F32, kind="ExternalInput")
    hal = nc.dram_tensor("hal", [128, 2], F32, kind="ExternalInput")
    out_d = nc.dram_tensor("out", [C, NCH, 512], F32, kind="ExternalOutput")

    with tile.TileContext(nc) as tc:
        with (
            tc.tile_pool(name="const", bufs=1) as cst,
            tc.tile_pool(name="wcb", bufs=4) as wcbp,
            tc.tile_pool(name="wct", bufs=4) as wctp,
            tc.tile_pool(name="fcwp", bufs=2) as fcwp,
            tc.tile_pool(name="cbcp", bufs=1) as cbcp,
            tc.tile_pool(name="ttp", bufs=2) as ttp,
            tc.tile_pool(name="xs", bufs=2) as xsp,
            tc.tile_pool(name="gb", bufs=2) as gbp,
            tc.tile_pool(name="xn", bufs=2) as xnp,
            tc.tile_pool(name="ot", bufs=2) as otp,
            tc.tile_pool(name="pmain", bufs=2, space="PSUM") as pmain,
            tc.tile_pool(name="paux", bufs=2, space="PSUM") as paux,
            tc.tile_pool(name="gpsp", bufs=3, space="PSUM") as gpsp,
        ):
            # ---- tiny constants (sync queue head) -----------------------
            u5r = cst.tile([45, 45], F32R)
            nc.sync.dma_start(out=u5r[:], in_=u5[:].bitcast(F32R))
            id_t = cst.tile([128, 128], F32)
            nc.sync.dma_start(out=id_t[:], in_=ident[:])
            id_r = cst.tile([128, 128], F32R)
            nc.sync.dma_start(out=id_r[:], in_=ident[:].bitcast(F32R))
            sswf = cst.tile([NH, 27], F32)
            nc.sync.dma_start(out=sswf[:], in_=ssw[:])
            graw = cst.tile([128, 1], F32)
            nc.sync.dma_start(out=graw[:], in_=bg[:].to_broadcast((128, 1)))
            braw = cst.tile([128, 1], F32)
            nc.sync.dma_start(out=braw[:], in_=bb[:].to_broadcast((128, 1)))
            convb = cst.tile([128, 1], F32)
            nc.sync.dma_start(out=convb[0:64, :], in_=cgb[:])
            nc.sync.dma_start(out=convb[64:128, :], in_=cbb[:])
            spadeb = cst.tile([128, 1], F32)
            nc.sync.dma_start(out=spadeb[0:64, :], in_=sgbb[:])
            nc.sync.dma_start(out=spadeb[64:128, :], in_=sbbb[:])
            ssb_t = cst.tile([NH, 1], F32)
            nc.sync.dma_start(out=ssb_t[:], in_=ssb[:])
            hal_t = cst.tile([128, 2], F32)
            nc.sync.dma_start(out=hal_t[:], in_=hal[:])
            fcbt_sb = cst.tile([128, 4, F], F32)
            for kb in range(4):
                nc.sync.dma_start(out=fcbt_sb[:, kb, :],
                                  in_=fcbt[kb * 128:(kb + 1) * 128, :])

            ones_t = cst.tile([128, 1], F32)
            nc.gpsimd.memset(ones_t[:], 1.0)
            eps_t = cst.tile([C, 1], F32)
            nc.gpsimd.memset(eps_t[:], 1e-5)
            half1 = cst.tile([128, 1], F32)
            nc.gpsimd.memset(half1[0:64, :], 1.0)
            nc.gpsimd.memset(half1[64:128, :], 0.0)
            zsb = cst.tile([128, 132], F32)
            nc.gpsimd.memset(zsb[:], 0.0)

            # blending factors
            gsig = cst.tile([128, 1], F32)
            nc.scalar.activation(gsig[:], graw[:], AF.Sigmoid)
            bsig = cst.tile([128, 1], F32)
            nc.scalar.activation(bsig[:], braw[:], AF.Sigmoid)
            gba = cst.tile([128, 1], F32)
            nc.vector.tensor_copy(gba[0:64, :], gsig[0:64, :])
            nc.vector.tensor_copy(gba[64:128, :], bsig[64:128, :])
            om_gba = cst.tile([128, 1], F32)
            nc.scalar.activation(om_gba[:], gba[:], AF.Identity, bias=ones_t[:], scale=-1.0)
            tb1 = cst.tile([128, 1], F32)
            nc.vector.tensor_mul(tb1[:], convb[:], gba[:])
            tb2 = cst.tile([128, 1], F32)
            nc.vector.tensor_mul(tb2[:], spadeb[:], om_gba[:])
            bias_t = cst.tile([128, 1], F32)
            nc.vector.tensor_add(bias_t[:], tb1[:], tb2[:])
            bias1_t = cst.tile([128, 1], F32)
            nc.vector.tensor_add(bias1_t[:], bias_t[:], half1[:])

            # ---- big weight loads: wcb (PE-critical) then fw (scalar q) -
            wcbs = []
            for kb in range(4):
                wcb = wcbp.tile([128, 1152], F32, tag="wcb", name=f"wcb{kb}")
                nc.sync.dma_start(out=wcb[0:64, :], in_=cgw[:, kb * 1152:(kb + 1) * 1152])
                nc.sync.dma_start(out=wcb[64:128, :], in_=cbw[:, kb * 1152:(kb + 1) * 1152])
                wcbs.append(wcb)
            sgb = cst.tile([128, 1152], F32)
            nc.sync.dma_start(out=sgb[0:64, :], in_=sgw[:])
            nc.sync.dma_start(out=sgb[64:128, :], in_=sbw[:])

            # ---- grids: pre-shifted replicated loads (gpsimd queue) -----
            sel45 = cst.tile([45, SEG_N], F32R)
            segp = segg[:].ap[0][0]
            for ty in range(3):
                src = bass.AP(tensor=segg[:].tensor, offset=ty * GW,
                              ap=[[1, 3], [segp, F], [1, SEG_N]])
                nc.gpsimd.dma_start(out=sel45[15 * ty:15 * ty + 15, :],
                                    in_=src.bitcast(F32R))
            mask27 = cst.tile([27, MASK_N], F32R)
            maskp_ = maskg[:].ap[0][0]
            for ty in range(3):
                src = bass.AP(tensor=maskg[:].tensor, offset=ty * GW,
                              ap=[[1, 3], [maskp_, 3], [1, MASK_N]])
                nc.gpsimd.dma_start(out=mask27[9 * ty:9 * ty + 9, :],
                                    in_=src.bitcast(F32R))

            # ---- region masks part 1: cnt (PE) -> t (ACT, parked in SBUF)
            t_sb = cst.tile([45, SEG_N], dt.bfloat16)
            segchunks = []
            off = 0 if lvl >= 2 else SEG_N
            while off < SEG_N:
                n = min(512, SEG_N - off)
                segchunks.append((off, n))
                off += n
            for off, n in segchunks:
                pc = paux.tile([45, 512], F32, tag="aux")
                nc.tensor.matmul(pc[:, 0:n], u5r[:], sel45[:, off:off + n],
                                 start=True, stop=True)
                nc.scalar.activation(t_sb[:, off:off + n], pc[:, 0:n], AF.Relu,
                                     bias=ones_t[0:45, :], scale=-1.0)

            # ---- shared conv (mask 3 -> NH), pre-shifted rows -----------
            if lvl >= 3:
                ptp = paux.tile([27, 128], F32, tag="aux")
                nc.tensor.transpose(ptp[:], sswf[:], id_t[:])
                sswT = cst.tile([27, 128], F32R)
                nc.scalar.activation(sswT[:], ptp[:], AF.Copy)

                actv = cst.tile([NH, SR, GW], F32R)
                bord = actv[:, :, 0:1]
                nc.vector.tensor_copy(
                    bass.AP(tensor=bord.tensor, offset=bord.offset,
                            ap=[bord.ap[0], [GW, SR], [GW - 1, 2]]),
                    zsb[:].rearrange("p (a b) -> p a b", a=SR))
                m3 = mask27[:].rearrange("p (r c) -> p r c", c=GW)
                for a in range(ACH):
                    r = 3 * a
                    psh = paux.tile([NH, 3, 128], F32, tag="aux")
                    nc.tensor.matmul(psh[:], sswT[:], m3[:, r:r + 3, 0:128],
                                     start=True, stop=True)
                    nc.scalar.activation(actv[:, r:r + 3, 1:129], psh[:], AF.Relu,
                                         bias=ssb_t[:], scale=1.0)

            # ---- spade gamma/beta lhsT ----------------------------------
            if lvl >= 6:
                nc.vector.tensor_scalar_mul(sgb[:], sgb[:], om_gba[:])
                spT = cst.tile([128, 9, 128], F32R)
                sgb3 = sgb[:].rearrange("p (l t) -> p l t", t=9)
                for t in range(9):
                    pt = paux.tile([128, 128], F32, tag="aux")
                    nc.tensor.transpose(pt[:], sgb3[:, :, t], id_t[:])
                    nc.scalar.activation(spT[:, t, :], pt[:], AF.Copy)

            # ---- mu path (fw on scalar queue; muls split DVE/Pool) ------
            if lvl >= 4:
                z_sb = cst.tile([128, 4, F], F32)
                muT = cst.tile([128, 4, F], F32R)
                for j in range(F):
                    cbc = cbcp.tile([128, L], F32, tag="cbc")
                    nc.scalar.dma_start(out=cbc[:],
                                        in_=codes[j:j + 1, :].to_broadcast((128, L)))
                    eng = nc.vector if j < 3 else nc.gpsimd
                    for kb in range(4):
                        fw = fcwp.tile([128, L], F32, tag="fcw")
                        nc.scalar.dma_start(out=fw[:], in_=fcw[j, kb * 128:(kb + 1) * 128, :])
                        tts = ttp.tile([128, L], F32, tag="tts")
                        eng.tensor_mul(tts[:], fw[:], cbc[:])
                        nc.vector.reduce_sum(out=z_sb[:, kb, j:j + 1], in_=tts[:],
                                             axis=mybir.AxisListType.X)
                for kb in range(4):
                    nc.vector.tensor_add(z_sb[:, kb, :], z_sb[:, kb, :],
                                         fcbt_sb[:, kb, :])
                for kb in range(4):
                    nc.scalar.activation(muT[:, kb, :], z_sb[:, kb, :], AF.Relu)

            # ---- wct transposes (PE) + copies (ACT) ---------------------
            if lvl >= 5:
                wcts = []
                for kb in range(4):
                    wcb = wcbs[kb]
                    nc.vector.tensor_scalar_mul(wcb[:], wcb[:], gba[:])
                    wct = wctp.tile([128, 9, 128], F32R, tag="wct", name=f"wct{kb}")
                    wcb3 = wcb[:].rearrange("p (l t) -> p l t", t=9)
                    for t in range(9):
                        pt = paux.tile([128, 128], F32, tag="aux")
                        nc.tensor.transpose(pt[:], wcb3[:, :, t], id_t[:])
                        nc.scalar.activation(wct[:, t, :], pt[:], AF.Copy)
                    wcts.append(wct)

            # ---- region masks part 2: sel = seg * t (DVE, after mu) -----
            for off, n in segchunks:
                nc.vector.tensor_mul(sel45[:, off:off + n],
                                     sel45[:, off:off + n].bitcast(F32),
                                     t_sb[:, off:off + n])
            if lvl >= 3:
                nc.vector.tensor_scalar_mul(actv[:, 0, :], actv[:, 0, :].bitcast(F32),
                                            hal_t[:, 0:1])
                nc.vector.tensor_scalar_mul(actv[:, SR - 1, :], actv[:, SR - 1, :].bitcast(F32),
                                            hal_t[:, 1:2])

            # ---- G matmuls -> selG --------------------------------------
            if lvl >= 5:
                gps = [gpsp.tile([F, 3, 128], F32, tag="gps", name=f"gps{_g}")
                       for _g in range(3)]
                for kb in range(4):
                    for g in range(3):
                        nc.tensor.matmul(gps[g][:], muT[:, kb, :],
                                         wcts[kb][:, 3 * g:3 * g + 3, :],
                                         start=(kb == 0), stop=(kb == 3))
                selG = cst.tile([45, 128], F32R)
                gstage = cst.tile([F, 9, 128], F32)
                for g in range(3):
                    nc.scalar.activation(gstage[:, 3 * g:3 * g + 3, :], gps[g][:], AF.Copy)
                for t in range(9):
                    nc.sync.dma_start(out=selG[F * t:F * t + F, :],
                                      in_=gstage[:, t, :].bitcast(F32R))

            # ---- instance-norm stats (xb on sync queue) -----------------
            if lvl >= 7:
                stats_t = cst.tile([C, 32, 6], F32)
                for q in range(16):
                    xt = xsp.tile([C, 2, 512], F32, tag="xs")
                    nc.sync.dma_start(out=xt[:], in_=xb[:, q * 1024:(q + 1) * 1024]
                                      .rearrange("c (k n) -> c k n", k=2))
                    for k in range(2):
                        nc.vector.bn_stats(out=stats_t[:, 2 * q + k, :], in_=xt[:, k, :])
                mv = cst.tile([C, 2], F32)
                nc.vector.bn_aggr(out=mv[:], in_=stats_t[:])
                sd = cst.tile([C, 1], F32)
                nc.scalar.activation(sd[:], mv[:, 1:2], AF.Sqrt, bias=eps_t[:], scale=1.0)
                rstd = cst.tile([C, 1], F32)
                nc.vector.reciprocal(rstd[:], sd[:])
                nbias = cst.tile([C, 1], F32)
                nc.vector.tensor_mul(nbias[:], mv[:, 0:1], rstd[:])
                nc.vector.tensor_scalar_mul(nbias[:], nbias[:], -1.0)

            # ---- main conv + epilogue (epilogue one chunk behind) -------
            if lvl >= 8:
                s3 = sel45[:].rearrange("p (r c) -> p r c", c=GW)
                xt2s, xnts, pms = {}, {}, {}

                def conv_chunk(i):
                    xt2s[i] = xnp.tile([C, 4, 128], F32, tag="xn", name=f"xt2_{i}")
                    nc.gpsimd.dma_start(out=xt2s[i][:],
                                        in_=xown[:, i * 512:(i + 1) * 512].rearrange(
                                            "c (r w) -> c r w", r=4))
                    xnts[i] = otp.tile([C, 4, 128], F32, tag="ot", name=f"xnt_{i}")
                    pm = pmain.tile([128, 4, 128], F32, tag="pm", name=f"pm_{i}")
                    pms[i] = pm
                    for t in range(9):
                        ty, tx = divmod(t, 3)
                        nc.tensor.matmul(pm[:], spT[:, t, :],
                                         actv[:, 4 * i + ty:4 * i + ty + 4, tx:tx + 128],
                                         start=(t == 0), stop=False)
                    nc.tensor.matmul(pm[:], selG[:], s3[:, 4 * i:4 * i + 4, 0:128],
                                     start=False, stop=True)

                def epi_chunk(i):
                    pm = pms.pop(i)
                    gb = gbp.tile([128, 4, 128], F32R, tag="gb", name=f"gb_{i}")
                    nc.scalar.activation(gb[:], pm[:], AF.Identity,
                                         bias=bias1_t[:], scale=1.0)
                    pb = gpsp.tile([64, 4, 128], F32, tag="gps", name=f"pb_{i}")
                    nc.tensor.matmul(pb[:].rearrange("p t c -> p (t c)"), id_r[:, 64:128],
                                     gb[:].rearrange("p t c -> p (t c)"),
                                     start=True, stop=True)
                    xt2, xnt = xt2s.pop(i), xnts[i]
                    nc.gpsimd.tensor_scalar(xnt[:], xt2[:],
                                            rstd[:], nbias[:],
                                            op0=ALU.mult, op1=ALU.add)
                    nc.gpsimd.tensor_mul(xnt[:], xnt[:], gb[0:64, :, :].bitcast(F32))
                    nc.vector.tensor_add(xnt[:].rearrange("p t c -> p (t c)"),
                                         xnt[:].rearrange("p t c -> p (t c)"),
                                         pb[:].rearrange("p t c -> p (t c)"))
                    nc.sync.dma_start(out=out_d[:, i, :],
                                      in_=xnts.pop(i)[:].rearrange("c r w -> c (r w)"))

                conv_chunk(0)
                for i in range(1, NCH):
                    conv_chunk(i)
                    epi_chunk(i - 1)
                epi_chunk(NCH - 1)

    nc.finalize()
    return nc


_NC = None


def kernel(**inputs):
    global _NC
    x = np.asarray(inputs["x"], dtype=np.float32)
    segmap = np.asarray(inputs["segmap"], dtype=np.float32)
    codes_vector = np.asarray(inputs["codes_vector"], dtype=np.float32)
    mask = np.asarray(inputs["mask"], dtype=np.float32)
    fc_w = np.ascontiguousarray(np.asarray(inputs["fc_w"], dtype=np.float32))
    fc_b = np.asarray(inputs["fc_b"], dtype=np.float32)
    conv_gamma_w = np.asarray(inputs["conv_gamma_w"], dtype=np.float32)
    conv_gamma_b = np.asarray(inputs["conv_gamma_b"], dtype=np.float32)
    conv_beta_w = np.asarray(inputs["conv_beta_w"], dtype=np.float32)
    conv_beta_b = np.asarray(inputs["conv_beta_b"], dtype=np.float32)
    spade_shared_w = np.asarray(inputs["spade_shared_w"], dtype=np.float32)
    spade_shared_b = np.asarray(inputs["spade_shared_b"], dtype=np.float32)
    spade_gamma_w = np.asarray(inputs["spade_gamma_w"], dtype=np.float32)
    spade_gamma_b = np.asarray(inputs["spade_gamma_b"], dtype=np.float32)
    spade_beta_w = np.asarray(inputs["spade_beta_w"], dtype=np.float32)
    spade_beta_b = np.asarray(inputs["spade_beta_b"], dtype=np.float32)
    blending_gamma = np.asarray(inputs["blending_gamma"], dtype=np.float32)
    blending_beta = np.asarray(inputs["blending_beta"], dtype=np.float32)

    if _NC is None:
        _NC = _build_nc()

    shared = {
        "fcw": np.ascontiguousarray(fc_w),
        "fcbt": np.ascontiguousarray(fc_b.T),
        "cgw": np.ascontiguousarray(conv_gamma_w.reshape(C, L * 9)),
        "cbw": np.ascontiguousarray(conv_beta_w.reshape(C, L * 9)),
        "sgw": np.ascontiguousarray(spade_gamma_w.reshape(C, NH * 9)),
        "sbw": np.ascontiguousarray(spade_beta_w.reshape(C, NH * 9)),
        "ssw": np.ascontiguousarray(spade_shared_w.transpose(0, 2, 3, 1).reshape(NH, 27)),
        "cgb": conv_gamma_b.reshape(C, 1), "cbb": conv_beta_b.reshape(C, 1),
        "sgbb": spade_gamma_b.reshape(C, 1), "sbbb": spade_beta_b.reshape(C, 1),
        "ssb": spade_shared_b.reshape(NH, 1),
        "bg": blending_gamma.reshape(1, 1), "bb": blending_beta.reshape(1, 1),
        "u5": np.kron(np.eye(9, dtype=np.float32), np.tril(np.ones((F, F), np.float32), -1)),
        "ident": np.eye(128, dtype=np.float32),
        "zz": np.zeros((128, 652), np.float32),
    }

    in_maps = []
    for c in range(NCORES):
        b, half = divmod(c, 2)
        h0 = half * ROWS
        segp = np.zeros((F, SR * GW + 264), np.float32).reshape(F, -1)
        segp2 = np.zeros((F, SR, GW), np.float32)
        r_lo, r_hi = h0 - 1, h0 + ROWS + 1  # exclusive
        s_lo, s_hi = max(r_lo, 0), min(r_hi, H)
        segp2[:, s_lo - r_lo:s_hi - r_lo, 1:129] = segmap[b, :, s_lo:s_hi, :]
        segp[:, 0:SR * GW] = segp2.reshape(F, -1)
        maskp = np.zeros((3, MR * GW + 264), np.float32)
        maskp2 = np.zeros((3, MR, GW), np.float32)
        m_lo, m_hi = h0 - 2, h0 + ROWS + 2
        ms_lo, ms_hi = max(m_lo, 0), min(m_hi, H)
        maskp2[:, ms_lo - m_lo:ms_hi - m_lo, 1:129] = mask[b, :, ms_lo:ms_hi, :]
        maskp[:, 0:MR * GW] = maskp2.reshape(3, -1)
        in_maps.append(dict(
            shared,
            xb=np.ascontiguousarray(x[b].reshape(C, H * W)),
            xown=np.ascontiguousarray(x[b, :, h0:h0 + ROWS, :].reshape(C, ROWS * W)),
            hal=np.ones((128, 2), np.float32) * np.array(
                [0.0 if h0 == 0 else 1.0, 0.0 if h0 + ROWS == H else 1.0],
                np.float32)[None, :],
            segg=np.ascontiguousarray(segp),
            maskg=np.ascontiguousarray(maskp),
            codes=np.ascontiguousarray(codes_vector[b]),
        ))

    res = run_bass_kernel_spmd(_NC, in_maps, list(range(NCORES)))

    out = np.empty((B, C, H, W), np.float32)
    for c in range(NCORES):
        b, half = divmod(c, 2)
        h0 = half * ROWS
        out[b, :, h0:h0 + ROWS, :] = res.results[c]["out"].reshape(C, ROWS, W)
    return out



# revision 21
# speedup vs baseline: 2.0023x; 2.0023x over previous
"""Trainium2 Bass kernel for nn_Decoder_22196390985918 (SPADE-style decoder).

Sharding: 8 cores = (batch b in 0..3) x (H-half in 0..1). Each core computes
out[b, :, h0:h0+64, :] for h0 = 64*(core%2).

v2 restructure vs baseline:
- All weights host-side pre-transposed into final lhsT layouts, blend
  (sigmoid) factors folded in on host, cast to bf16, packed into 2 big
  DMA blobs + 1 small fp32 const blob -> ~6 input DMAs total instead of ~60.
- x loaded once as bf16 [128, 8192]: partitions 0-63 = own 64 rows
  (channel-major, reused by the epilogue), 64-127 = other half (stats only).
  Instance-norm stats via one multi-chunk bn_stats; halves combined with
  plain [64]-partition tensor ops (no transposes / DMAs).
- Region-priority mask fused: sel = (cnt == 0) * seg as one DVE
  scalar_tensor_tensor reading cnt straight from PSUM.
- sel45 partition order (j, ty, tx) so the G table rearrange is ONE
  SBUF->SBUF DMA. u5 = kron(tril, eye(9)) accordingly.
- Epilogue: out = (x - mu) * A + B with A = (psum_g + bias_g)*rstd via one
  ACT op, B = psum_b + bias_b via another; no PE broadcast matmul.
- Main conv loop starts as soon as selG is ready (~20us) and overlaps all
  remaining DMA.
"""
import numpy as np
import ml_dtypes

import concourse.bacc as bacc
import concourse.bass as bass
import concourse.mybir as mybir
import concourse.tile as tile
from concourse.bass_utils import run_bass_kernel_spmd

dt = mybir.dt
F32 = dt.float32
BF16 = dt.bfloat16
AF = mybir.ActivationFunctionType
ALU = mybir.AluOpType
BF = ml_dtypes.bfloat16

B, C, H, W, F, L, NH = 4, 64, 128, 128, 5, 512, 128
GW = 130                    # padded grid width  (image col = grid col - 1)
SR = 66                     # seg/sel/actv grid rows (image row = h0 - 1 + r)
MR = 68                     # mask grid rows (image row = h0 - 2 + r)
SEG_N = SR * GW             # 8580
MASK_N = MR * GW            # 8840
GLS = SEG_N + 48            # seg grid line length (incl. u5 tail)
GLM = MASK_N                # mask grid line length
ROWS = 64                   # output rows per core
NCH = 16                    # main conv chunks (4 rows x 128 cols, N=512)
ACH = 22                    # shared conv chunks (3 rows x 128 cols, N=384)
NCORES = 8

# bigwa layout (per-partition elem offsets): fcw [5*4*512] then codes [5*512]
OFF_FCW = 0
OFF_CODES = 5 * 4 * 512     # 10240
BWA = OFF_CODES + 5 * 512   # 12800
# bigwb layout: wct [4*9*128] then spT [9*128] then sswT [128]
OFF_WCT = 0
OFF_SPT = 4 * 9 * 128       # 4608
OFF_SSW = OFF_SPT + 9 * 128  # 5760
BWB = OFF_SSW + 128         # 5888
# constf layout (fp32): 0 biasg, 1 ssb, 2-3 hal, 4-23 fcbt, 24 biasb64
CF = 25
# u5 lives in the grids blob tail on partitions 0..44
OFF_U5 = SEG_N              # 8580..8625


def _build_nc():
    nc = bacc.Bacc()

    gseg_d = nc.dram_tensor("gseg", [45, GLS], BF16, kind="ExternalInput")
    gmask_d = nc.dram_tensor("gmask", [27, GLM], BF16, kind="ExternalInput")
    bigwa_d = nc.dram_tensor("bigwa", [128, BWA], BF16, kind="ExternalInput")
    bigwb_d = nc.dram_tensor("bigwb", [128, BWB], BF16, kind="ExternalInput")
    constf_d = nc.dram_tensor("constf", [128, CF], F32, kind="ExternalInput")
    xb2_d = nc.dram_tensor("xb2", [128, 8192], BF16, kind="ExternalInput")
    out_d = nc.dram_tensor("out", [C, 4, 4 * 512], F32, kind="ExternalOutput")

    with tile.TileContext(nc) as tc:
        with (
            tc.tile_pool(name="const", bufs=1) as cst,
            tc.tile_pool(name="mus", bufs=2) as musp,
            tc.tile_pool(name="gb", bufs=3) as gbp,
            tc.tile_pool(name="ep", bufs=3) as epp,
            tc.tile_pool(name="ot", bufs=2) as otp,
            tc.tile_pool(name="pmain", bufs=3, space="PSUM") as pmain,
            tc.tile_pool(name="paux", bufs=2, space="PSUM") as paux,
            tc.tile_pool(name="pg", bufs=2, space="PSUM") as pgp,
        ):
            # ---- input DMAs ---------------------------------------------
            gseg = cst.tile([45, GLS], BF16)
            nc.sync.dma_start(out=gseg[:], in_=gseg_d[:])
            gmask = cst.tile([27, GLM], BF16)
            nc.sync.dma_start(out=gmask[:], in_=gmask_d[:])
            bigwa = cst.tile([128, BWA], BF16)
            nc.gpsimd.dma_start(out=bigwa[:], in_=bigwa_d[:])
            bigwb = cst.tile([128, BWB], BF16)
            nc.sync.dma_start(out=bigwb[:], in_=bigwb_d[:])
            constf = cst.tile([128, CF], F32)
            nc.gpsimd.dma_start(out=constf[:], in_=constf_d[:])
            xb2 = cst.tile([128, 8192], BF16)
            nc.sync.dma_start(out=xb2[:], in_=xb2_d[:])

            sel45 = gseg[:, 0:SEG_N]
            u5 = gseg[:, OFF_U5:OFF_U5 + 45]
            mask27 = gmask[:, 0:MASK_N]
            epst = cst.tile([128, 1], F32)
            nc.gpsimd.memset(epst[:], 1e-5)
            zt = cst.tile([128, 1], F32)
            nc.gpsimd.memset(zt[:], 0.0)
            biasg = constf[:, 0:1]
            ssb = constf[:, 1:2]
            hal = constf[:, 2:4]
            fcbt = constf[:, 4:24].rearrange("p (k j) -> p k j", k=4)
            biasb64 = constf[0:64, 24:25]

            # ---- region masks: cnt (PE) -> sel = (cnt==0)*seg (DVE) -----
            segchunks = []
            off = 0
            while off < SEG_N:
                n = min(512, SEG_N - off)
                segchunks.append((off, n))
                off += n
            for off, n in segchunks:
                pc = paux.tile([45, 512], F32, tag="aux")
                nc.tensor.matmul(pc[:, 0:n], u5, sel45[:, off:off + n],
                                 start=True, stop=True)
                nc.vector.scalar_tensor_tensor(
                    out=sel45[:, off:off + n], in0=pc[:, 0:n], scalar=0.0,
                    in1=sel45[:, off:off + n], op0=ALU.is_equal, op1=ALU.mult)

            # ---- shared conv (mask 3 -> NH) + actv assembly -------------
            actv = cst.tile([NH, SR, GW], BF16)
            bord = actv[:, :, 0:1]
            nc.gpsimd.memset(
                bass.AP(tensor=bord.tensor, offset=bord.offset,
                        ap=[bord.ap[0], [GW, SR], [GW - 1, 2]]), 0.0)
            sswT = bigwb[0:27, OFF_SSW:OFF_SSW + 128]
            m3 = mask27.rearrange("p (r c) -> p r c", c=GW)
            for a in range(ACH):
                r = 3 * a
                psh = paux.tile([NH, 3, 128], F32, tag="aux")
                nc.tensor.matmul(psh[:], sswT, m3[:, r:r + 3, 0:128],
                                 start=True, stop=True)
                nc.scalar.activation(actv[:, r:r + 3, 1:129], psh[:], AF.Relu,
                                     bias=ssb, scale=1.0)
            nc.vector.tensor_scalar_mul(actv[:, 0, :], actv[:, 0, :], hal[:, 0:1])
            nc.vector.tensor_scalar_mul(actv[:, SR - 1, :], actv[:, SR - 1, :],
                                        hal[:, 1:2])

            # ---- mu path: z = fcb + sum_l fcw*codes (DVE TTR), relu -----
            z_sb = cst.tile([128, 4, F], F32)
            for j in range(F):
                cview = bigwa[:, OFF_CODES + j * L:OFF_CODES + (j + 1) * L]
                for kb in range(4):
                    fview = bigwa[:, OFF_FCW + (j * 4 + kb) * L:
                                  OFF_FCW + (j * 4 + kb + 1) * L]
                    mus = musp.tile([128, L], BF16, tag="mus")
                    nc.vector.scalar_tensor_tensor(
                        out=mus[:], in0=fview, scalar=1.0, in1=cview,
                        op0=ALU.mult, op1=ALU.mult,
                        accum_out=z_sb[:, kb, j:j + 1])
            nc.vector.tensor_add(z_sb[:], z_sb[:], fcbt[:])
            muT = cst.tile([128, 4, F], BF16)
            nc.scalar.activation(muT[:], z_sb[:], AF.Relu, bias=zt[:])

            # ---- G tables -> selG (one rearrange DMA) -------------------
            gstage = cst.tile([F, 9, 128], BF16)
            for g in range(3):
                gps = pgp.tile([F, 3, 128], F32, tag="gps")
                for kb in range(4):
                    wview = bigwb[:, OFF_WCT + kb * 1152 + g * 384:
                                  OFF_WCT + kb * 1152 + (g + 1) * 384]
                    nc.tensor.matmul(gps[:], muT[:, kb, :],
                                     wview.rearrange("p (t c) -> p t c", t=3),
                                     start=(kb == 0), stop=(kb == 3))
                nc.scalar.activation(gstage[:, 3 * g:3 * g + 3, :], gps[:], AF.Copy)
            selG = cst.tile([45, 128], BF16)
            nc.scalar.dma_start(out=selG[:], in_=gstage[:])

            # ---- instance-norm stats ------------------------------------
            st = cst.tile([128, 16, 6], F32)
            for q in range(16):
                nc.vector.bn_stats(out=st[:, q, :],
                                   in_=xb2[:, q * 512:(q + 1) * 512])
            mv = cst.tile([128, 2], F32)
            nc.vector.bn_aggr(out=mv[:], in_=st[:])
            mvm = cst.tile([128, 2], F32)
            nc.vector.tensor_copy(mvm[:, 0:1], mv[:, 0:1])
            nc.vector.scalar_tensor_tensor(
                out=mvm[:, 1:2], in0=mv[:, 0:1], scalar=mv[:, 0:1], in1=mv[:, 1:2],
                op0=ALU.mult, op1=ALU.add)
            oth = cst.tile([64, 2], F32)
            nc.vector.stream_shuffle(oth[:], mvm[64:128, :], list(range(32)))
            mus_ = cst.tile([64, 1], F32)
            nc.vector.tensor_add(mus_[:], mvm[0:64, 0:1], oth[:, 0:1])
            m2s = cst.tile([64, 1], F32)
            nc.vector.tensor_add(m2s[:], mvm[0:64, 1:2], oth[:, 1:2])
            muc = cst.tile([64, 1], F32)
            nc.vector.tensor_scalar_mul(muc[:], mus_[:], 0.5)
            mu2 = cst.tile([64, 1], F32)
            nc.vector.tensor_mul(mu2[:], muc[:], muc[:])
            varc = cst.tile([64, 1], F32)
            nc.vector.scalar_tensor_tensor(
                out=varc[:], in0=m2s[:], scalar=0.5, in1=mu2[:],
                op0=ALU.mult, op1=ALU.subtract)
            sd = cst.tile([64, 1], F32)
            nc.scalar.activation(sd[:], varc[:], AF.Sqrt, bias=epst[0:64, :])
            rstd = cst.tile([64, 1], F32)
            nc.vector.reciprocal(rstd[:], sd[:])
            biasgr = cst.tile([64, 1], F32)
            nc.vector.tensor_mul(biasgr[:], biasg[0:64, :], rstd[:])

            # ---- main conv + epilogue (epilogue one chunk behind) -------
            s3 = sel45.rearrange("p (r c) -> p r c", c=GW)
            a3 = actv[:]
            pms = {}
            ots = {}

            def conv_chunk(i):
                pm = pmain.tile([128, 4, 128], F32, tag="pm", name=f"pm_{i}")
                pms[i] = pm
                for t in range(9):
                    ty, tx = divmod(t, 3)
                    nc.tensor.matmul(
                        pm[:], bigwb[:, OFF_SPT + t * 128:OFF_SPT + (t + 1) * 128],
                        a3[:, 4 * i + ty:4 * i + ty + 4, tx:tx + 128],
                        start=(t == 0), stop=False)
                nc.tensor.matmul(pm[:], selG[:], s3[:, 4 * i:4 * i + 4, 0:128],
                                 start=False, stop=True)

            def epi_chunk(i):
                pm = pms.pop(i)
                if i % 4 == 0:
                    ots[i // 4] = otp.tile([64, 4, 4, 128], F32, tag="ot",
                                           name=f"ot_{i // 4}")
                ot = ots[i // 4]
                gt = gbp.tile([128, 4, 128], F32, tag="gb")
                nc.scalar.activation(gt[0:64, :, :], pm[0:64, :, :], AF.Identity,
                                     bias=biasgr[:], scale=rstd[:])
                nc.scalar.activation(gt[64:128, :, :], pm[64:128, :, :],
                                     AF.Identity, bias=biasg[64:128, :], scale=1.0)
                bbm = epp.tile([64, 4, 128], F32, tag="ep")
                nc.vector.stream_shuffle(bbm[:], gt[64:128, :, :], list(range(32)))
                xa = epp.tile([64, 4, 128], F32, tag="ep")
                nc.vector.scalar_tensor_tensor(
                    out=xa[:], in0=xb2[0:64, i * 512:(i + 1) * 512].rearrange(
                        "p (r w) -> p r w", r=4),
                    scalar=muc[:], in1=gt[0:64, :, :], op0=ALU.subtract,
                    op1=ALU.mult)
                nc.gpsimd.tensor_add(ot[:, i % 4, :, :], xa[:], bbm[:])
                if i % 4 == 3:
                    q = i // 4
                    nc.sync.dma_start(
                        out=out_d[:, q, :],
                        in_=ots.pop(q)[:].rearrange("c k r w -> c (k r w)"))

            conv_chunk(0)
            for i in range(1, NCH):
                conv_chunk(i)
                epi_chunk(i - 1)
            epi_chunk(NCH - 1)

    nc.finalize()
    return nc


_NC = None


def kernel(**inputs):
    global _NC
    x = np.asarray(inputs["x"], dtype=np.float32)
    segmap = np.asarray(inputs["segmap"], dtype=np.float32)
    codes_vector = np.asarray(inputs["codes_vector"], dtype=np.float32)
    mask = np.asarray(inputs["mask"], dtype=np.float32)
    fc_w = np.asarray(inputs["fc_w"], dtype=np.float32)
    fc_b = np.asarray(inputs["fc_b"], dtype=np.float32)
    cgw = np.asarray(inputs["conv_gamma_w"], dtype=np.float32)
    cgb = np.asarray(inputs["conv_gamma_b"], dtype=np.float32)
    cbw = np.asarray(inputs["conv_beta_w"], dtype=np.float32)
    cbb = np.asarray(inputs["conv_beta_b"], dtype=np.float32)
    ssw = np.asarray(inputs["spade_shared_w"], dtype=np.float32)
    ssb = np.asarray(inputs["spade_shared_b"], dtype=np.float32)
    sgw = np.asarray(inputs["spade_gamma_w"], dtype=np.float32)
    sgb = np.asarray(inputs["spade_gamma_b"], dtype=np.float32)
    sbw = np.asarray(inputs["spade_beta_w"], dtype=np.float32)
    sbb = np.asarray(inputs["spade_beta_b"], dtype=np.float32)
    bg = float(np.asarray(inputs["blending_gamma"]).reshape(-1)[0])
    bb_ = float(np.asarray(inputs["blending_beta"]).reshape(-1)[0])

    if _NC is None:
        _NC = _build_nc()

    ga = 1.0 / (1.0 + np.exp(-bg))
    ba = 1.0 / (1.0 + np.exp(-bb_))

    # bigwb: wct | spT | sswT  (shared across cores)
    bigwb = np.zeros((128, BWB), np.float32)
    # wct[p, kb*1152 + (3ty+tx)*128 + cc] = blend * conv_w[cc, kb*128+p, ty, tx]
    cw = np.concatenate([cgw * ga, cbw * ba], axis=0)          # [128, 512, 3, 3]
    wct = cw.reshape(128, 4, 128, 9).transpose(2, 1, 3, 0)     # [p, kb, t, cc]
    bigwb[:, OFF_WCT:OFF_SPT] = wct.reshape(128, 4608)
    sw = np.concatenate([sgw * (1 - ga), sbw * (1 - ba)], axis=0)  # [128, NH, 3, 3]
    spT = sw.reshape(128, 128, 9).transpose(1, 2, 0)           # [nh, t, cc]
    bigwb[:, OFF_SPT:OFF_SSW] = spT.reshape(128, 1152)
    # sswT[9ty+3tx+c, nh] = ssw[nh, c, ty, tx]
    sswT = ssw.transpose(2, 3, 1, 0).reshape(27, 128)
    bigwb[0:27, OFF_SSW:OFF_SSW + 128] = sswT
    bigwb = bigwb.astype(BF)

    # constf: biasg | ssb | hal | fcbt (hal per-core, rest shared)
    constf_base = np.zeros((128, CF), np.float32)
    constf_base[0:64, 0] = ga * cgb + (1 - ga) * sgb + 1.0
    constf_base[64:128, 0] = ba * cbb + (1 - ba) * sbb
    constf_base[:, 1] = ssb
    constf_base[0:64, 24] = ba * cbb + (1 - ba) * sbb
    # fcbt[p, 4 + kb*5 + j] = fc_b[j, kb*128+p]
    constf_base[:, 4:24] = fc_b.T.reshape(4, 128, F).transpose(1, 0, 2).reshape(128, 20)

    # u5[9j'+t', 9j+t] = (j' > j) * (t'==t)
    u5 = np.kron(np.tril(np.ones((F, F), np.float32), -1), np.eye(9, dtype=np.float32))

    in_maps = []
    for c in range(NCORES):
        b, half = divmod(c, 2)
        h0 = half * ROWS

        # bigwa: fcw (k-partition, l-free) | codes broadcast (per-batch)
        bigwa = np.zeros((128, BWA), np.float32)
        # fcw_sec[p, (j*4+kb)*512 + l] = fc_w[j, kb*128+p, l]
        bigwa[:, OFF_FCW:OFF_CODES] = (
            fc_w.reshape(F, 4, 128, L).transpose(2, 0, 1, 3).reshape(128, 10240))
        bigwa[:, OFF_CODES:BWA] = np.broadcast_to(
            codes_vector[b].reshape(1, F * L), (128, F * L))

        # grids: sel45 (j,ty,tx order) + u5 | mask27
        gsegh = np.zeros((45, GLS), np.float32)
        gmaskh = np.zeros((27, GLM), np.float32)
        segp = np.zeros((F, SR + 2, GW + 2), np.float32)
        r_lo, r_hi = h0 - 1, h0 + ROWS + 1
        s_lo, s_hi = max(r_lo, 0), min(r_hi, H)
        segp[:, s_lo - r_lo:s_hi - r_lo, 1:129] = segmap[b, :, s_lo:s_hi, :]
        for j in range(F):
            for ty in range(3):
                for tx in range(3):
                    sh = segp[j].reshape(-1)[ty * (GW + 2) + tx:]
                    v = np.lib.stride_tricks.as_strided(
                        sh, (SR, GW), (4 * (GW + 2), 4)).copy()
                    # row r of shifted grid = segp[j, r+ty, tx:tx+GW]
                    gsegh[9 * j + 3 * ty + tx, 0:SEG_N] = v.reshape(-1)
        gsegh[:, OFF_U5:OFF_U5 + 45] = u5
        maskp = np.zeros((3, MR + 2, GW + 2), np.float32)
        m_lo, m_hi = h0 - 2, h0 + ROWS + 2
        ms_lo, ms_hi = max(m_lo, 0), min(m_hi, H)
        maskp[:, ms_lo - m_lo:ms_hi - m_lo, 1:129] = mask[b, :, ms_lo:ms_hi, :]
        for cc in range(3):
            for ty in range(3):
                for tx in range(3):
                    sh = maskp[cc].reshape(-1)[ty * (GW + 2) + tx:]
                    v = np.lib.stride_tricks.as_strided(
                        sh, (MR, GW), (4 * (GW + 2), 4)).copy()
                    gmaskh[9 * ty + 3 * tx + cc, 0:MASK_N] = v.reshape(-1)

        constf = constf_base.copy()
        constf[:, 2] = 0.0 if h0 == 0 else 1.0
        constf[:, 3] = 0.0 if h0 + ROWS == H else 1.0

        xb2 = np.concatenate([
            x[b, :, h0:h0 + ROWS, :].reshape(C, 8192),
            x[b, :, ROWS - h0:ROWS - h0 + ROWS, :].reshape(C, 8192)], axis=0)

        in_maps.append(dict(
            gseg=np.ascontiguousarray(gsegh.astype(BF)),
            gmask=np.ascontiguousarray(gmaskh.astype(BF)),
            bigwa=np.ascontiguousarray(bigwa.astype(BF)),
            bigwb=bigwb,
            constf=np.ascontiguousarray(constf),
            xb2=np.ascontiguousarray(xb2.astype(BF)),
        ))

    res = run_bass_kernel_spmd(_NC, in_maps, list(range(NCORES)))

    out = np.empty((B, C, H, W), np.float32)
    for c in range(NCORES):
        b, half = divmod(c, 2)
        h0 = half * ROWS
        out[b, :, h0:h0 + ROWS, :] = res.results[c]["out"].reshape(C, ROWS, W)
    return out
